# revision 29
# baseline (speedup 1.0000x reference)
"""Trainium2 Bass kernel for the IoU polygon loss (nn_IoUPolyLoss).

Full inputs in, full (scalar) output out. Internally shards the 512
polygons over 8 NeuronCores (64 each: core c -> batch c//2, k-range
64*(c%2)). Rasterization-free: per polygon and scanline, coverage is
the alternating sum of the sorted edge-crossing x-coordinates; the
host combines per-polygon areas into the final loss:
    inter = (area_p + area_g - area_xor) / 2
    union = (area_p + area_g + area_xor) / 2

Device layout per core: partition p = hh*64 + poly (h-half hh), free
dims (side s=2, edge v=16, hl=64); scanline h = hh*64 + hl.

v2 design notes (cost-model driven):
 - affine crossing form xint = py*A + B with per-edge A = dx/dy,
   B = x1 - y1*A (tiny precompute) -> 4 big DVE ops instead of 7
 - crossing mask on the Pool engine in parallel with the DVE chain,
   using bg = roll(ag) (y2 of edge v is y1 of edge v+1)
 - values shifted by -128 before sorting: masked slots (0.0) sort
   ABOVE all real crossings (which are negative) and cancel pairwise
   under the alternating signs, so no sentinel fixup is needed
 - sort-16 / merge-32 in fp16 (integers < 2048 are exact): 2x DVE
   throughput on min/max, 4x on copies; full-coverage rounds are
   ping-ponged (2 ops), sparse rounds run in place (3 ops)
 - final alternating sums via scalar_tensor_tensor accum_out: one
   instruction per area (host only ever needs ap+ag and ax)
"""
import sys

import numpy as np

try:
    import concourse.bass as bass
except ImportError:
    sys.path.insert(0, "/opt/trn_rl_repo")
    import concourse.bass as bass

import concourse.mybir as mybir
import concourse.tile as tile
import concourse.bacc as bacc
from concourse.bass_utils import run_bass_kernel_spmd

OP = mybir.AluOpType
F32 = mybir.dt.float32
F16 = mybir.dt.float16
I32 = mybir.dt.int32
F = np.float32

MAGIC = 12582912.0            # 1.5 * 2^23, RN-to-int trick for |x| < 2^22
KK = 0.49545454545454547      # 0.5 - 1/220 margin
M2 = MAGIC + 128.0            # exact in fp32
MAGICM100 = MAGIC - 100.0     # exact in fp32

N_CORES = 8

LAST_RESULTS = None           # BassKernelResults of the most recent run


def _batcher16_pairs():
    n = 16
    rounds = []
    p = 1
    while p < n:
        k = p
        while k >= 1:
            los = []
            j = k % p
            while j <= n - 1 - k:
                for i in range(0, min(k, n - j - k)):
                    if (i + j) // (2 * p) == (i + j + k) // (2 * p):
                        los.append(i + j)
                j += 2 * k
            rounds.append((k, los))
            k //= 2
        p *= 2
    return rounds


def _decompose(idxs):
    n = len(idxs)
    if n == 1:
        return [[1, 1]]
    d = idxs[1] - idxs[0]
    if all(idxs[i] == idxs[0] + i * d for i in range(n)):
        return [[d, n]]
    run = 1
    while run < n and idxs[run] == idxs[0] + run * d:
        run += 1
    assert n % run == 0, f"cannot decompose {idxs}"
    outer = idxs[::run]
    do = outer[1] - outer[0]
    for oi, o in enumerate(outer):
        assert o == outer[0] + oi * do
        for ii in range(run):
            assert idxs[oi * run + ii] == o + ii * d, f"cannot decompose {idxs}"
    return [[do, len(outer)], [d, run]]


def _view(tile_ap, offset, dims):
    return bass.AP(
        tile_ap.tensor,
        tile_ap.offset + offset,
        [list(tile_ap.ap[0])] + [[s, c] for s, c in dims],
    )


def _vdims(idxs, inner=64):
    """AP dims for a set of v-indices (times stride 64, hl inner)."""
    return [[s * 64, c] for s, c in _decompose(idxs)] + [[1, inner]]


def _build_core_kernel(tc, areas_dram, table, pidx, gverts):
    nc = tc.nc
    view = _view
    with tc.tile_pool(name="main", bufs=1) as pool:
        # ---------------- loads: separate tiles so the gt DMA is not
        # serialized behind the (conservatively-tracked) indirect gather
        pidx_sb = pool.tile([128, 1], I32, tag="pidx")
        nc.sync.dma_start(out=pidx_sb[:], in_=pidx)
        rawp = pool.tile([128, 32], F32, tag="rawp")
        nc.gpsimd.indirect_dma_start(
            out=rawp[:],
            out_offset=None,
            in_=table,
            in_offset=bass.IndirectOffsetOnAxis(ap=pidx_sb[:, :1], axis=0),
        )
        rawg = pool.tile([128, 32], F32, tag="rawg")
        nc.sync.dma_start(out=rawg[:], in_=gverts)

        # ---------------- trunc(x)+100 via round(x - 0.5*sgn(x)) + 100,
        # per side so each starts as soon as its input lands
        verts = pool.tile([128, 64], F32, tag="verts")   # (s, v, coord)
        for off, src in ((32, rawg), (0, rawp)):
            m = pool.tile([128, 32], F32, tag=f"m{off}")
            nc.vector.tensor_scalar(m[:], src[:], 0.0, None, OP.is_ge)
            sh = pool.tile([128, 32], F32, tag=f"sh{off}")
            nc.vector.tensor_scalar(sh[:], m[:], -1.0, 0.5, OP.mult, OP.add)
            u = pool.tile([128, 32], F32, tag=f"u{off}")
            nc.vector.tensor_tensor(u[:], src[:], sh[:], OP.add)
            nc.vector.tensor_scalar(verts[:, off:off + 32], u[:],
                                    MAGIC, MAGICM100, OP.add, OP.subtract)

        # ---------------- edge precompute: A = dx/dy', B = x1 - y1*A
        ver2 = pool.tile([128, 64], F32, tag="ver2")     # roll v by 1
        nc.vector.tensor_copy(view(ver2[:], 0, [(32, 2), (2, 15), (1, 2)]),
                              view(verts[:], 2, [(32, 2), (2, 15), (1, 2)]))
        nc.vector.tensor_copy(view(ver2[:], 30, [(32, 2), (1, 2)]),
                              view(verts[:], 0, [(32, 2), (1, 2)]))
        dall = pool.tile([128, 64], F32, tag="dall")     # (dx, dy) per edge
        nc.vector.tensor_tensor(dall[:], ver2[:], verts[:], OP.subtract)

        pk = [(16, 2), (1, 16)]          # packed (s, v) dims for [128,32]
        x1v = view(verts[:], 0, [(32, 2), (2, 16)])
        y1v = view(verts[:], 1, [(32, 2), (2, 16)])
        dxv = view(dall[:], 0, [(32, 2), (2, 16)])
        dyv = view(dall[:], 1, [(32, 2), (2, 16)])

        z = pool.tile([128, 32], F32, tag="z")
        nc.vector.tensor_scalar(view(z[:], 0, pk), dyv, 0.0, None, OP.is_equal)
        ds = pool.tile([128, 32], F32, tag="ds")
        nc.vector.tensor_tensor(view(ds[:], 0, pk), dyv, view(z[:], 0, pk),
                                OP.add)
        rt = pool.tile([128, 32], F32, tag="rt")
        nc.vector.reciprocal(rt[:], ds[:])
        At = pool.tile([128, 32], F32, tag="At")
        nc.vector.tensor_tensor(view(At[:], 0, pk),
                                view(rt[:], 0, pk), dxv, OP.mult)
        n1 = pool.tile([128, 32], F32, tag="n1")
        nc.vector.tensor_tensor(view(n1[:], 0, pk), y1v,
                                view(At[:], 0, pk), OP.mult)
        Bt = pool.tile([128, 32], F32, tag="Bt")
        nc.vector.scalar_tensor_tensor(view(Bt[:], 0, pk),
                                       view(n1[:], 0, pk), -1.0, x1v,
                                       OP.mult, OP.add)

        # ---------------- py = hh*64 + hl  (f32 [128, 64])
        hlq = pool.tile([128, 64], I32, tag="hlq")
        nc.gpsimd.iota(hlq[:], pattern=[[1, 64]], base=0, channel_multiplier=0)
        pid = pool.tile([128, 1], I32, tag="pid")
        nc.gpsimd.iota(pid[:], pattern=[[0, 1]], base=0, channel_multiplier=1)
        hh64 = pool.tile([128, 1], I32, tag="hh64")
        nc.vector.tensor_scalar(hh64[:], pid[:], 64, None, OP.bitwise_and)
        hh64f = pool.tile([128, 1], F32, tag="hh64f")
        nc.vector.tensor_copy(hh64f[:], hh64[:])
        hlf = pool.tile([128, 64], F32, tag="hlf")
        nc.vector.tensor_copy(hlf[:], hlq[:])
        pyf = pool.tile([128, 64], F32, tag="pyf")
        nc.vector.tensor_scalar(pyf[:], hlf[:], hh64f[:, :1], None, OP.add)

        # fp16 casts of y1 and py for the crossing-mask compares
        y1h = pool.tile([128, 32], F16, tag="y1h")
        nc.vector.tensor_copy(view(y1h[:], 0, pk), y1v)
        pyh = pool.tile([128, 64], F16, tag="pyh")
        nc.vector.tensor_copy(pyh[:], pyf[:])

        # ---------------- grid stage, free = (s2, v16, hl64) = 2048
        GD = [(1024, 2), (64, 16), (1, 64)]
        Ag = view(At[:], 0, [(16, 2), (1, 16), (0, 64)])
        pyg = view(pyf[:], 0, [(0, 2), (0, 16), (1, 64)])

        def gtile(tag, dt=F32):
            t = pool.tile([128, 2048], dt, tag=tag)
            return t, view(t[:], 0, GD)

        # flat (s*v, hl) views: scalar_tensor_tensor needs <=2 free dims
        GF = [(64, 32), (1, 64)]
        BF = [(1, 32), (0, 64)]              # per-edge smalls broadcast on hl

        # DVE fp16: ag = (y1 <= py); crg = ag != roll(ag)
        agt, _ = gtile("ag", F16)
        nc.vector.tensor_tensor(view(agt[:], 0, GF),
                                view(y1h[:], 0, BF),
                                view(pyh[:], 0, [(0, 32), (1, 64)]),
                                OP.is_le)
        crt, crg = gtile("cr", F16)
        nc.vector.tensor_tensor(view(crt[:], 0, [(1024, 2), (64, 15), (1, 64)]),
                                view(agt[:], 64, [(1024, 2), (64, 15), (1, 64)]),
                                view(agt[:], 0, [(1024, 2), (64, 15), (1, 64)]),
                                OP.not_equal)
        nc.vector.tensor_tensor(view(crt[:], 15 * 64, [(1024, 2), (1, 64)]),
                                view(agt[:], 0, [(1024, 2), (1, 64)]),
                                view(agt[:], 15 * 64, [(1024, 2), (1, 64)]),
                                OP.not_equal)

        # xint chain split into hl-halves so the Act-engine rounding of
        # half A overlaps the DVE computing half B (no DVE idle bubble):
        #   DVE: t1 = py*A ; xkk = (t1+KK)+B     Act: wr = xkk+MAGIC
        #   DVE: cg = (wr - (MAGIC+128)) * crg  -> fp16, shifted by -128
        t1, t1g = gtile("t1")
        xkk, xkkg = gtile("xkk")
        wr, wrg = gtile("wr")
        T0, T0g = gtile("T0", F16)
        T1, T1g = gtile("T1", F16)
        GH = [(64, 32), (1, 32)]             # one hl-half, flat (s*v, hl)
        for h in (0, 32):
            nc.vector.tensor_tensor(
                view(t1[:], h, [(1024, 2), (64, 16), (1, 32)]),
                view(pyf[:], h, [(0, 2), (0, 16), (1, 32)]),
                view(At[:], 0, [(16, 2), (1, 16), (0, 32)]), OP.mult)
            nc.vector.scalar_tensor_tensor(
                view(xkk[:], h, GH), view(t1[:], h, GH),
                KK, view(Bt[:], 0, [(1, 32), (0, 32)]), OP.add, OP.add)
            nc.scalar.activation(view(wr[:], h, GH), view(xkk[:], h, GH),
                                 mybir.ActivationFunctionType.Copy, bias=MAGIC)
        for h in (0, 32):
            nc.vector.scalar_tensor_tensor(view(T0[:], h, GH),
                                           view(wr[:], h, GH),
                                           M2, view(crt[:], h, GH),
                                           OP.subtract, OP.mult)

        # ---------------- sort-16 along v (ascending), fp16
        # ping-pong when idle < np, else in-place; track current buffer
        tmp16 = pool.tile([128, 1024], F16, tag="tmp16")
        bufs = [T0, T1]
        cur = 0
        for k, los in _batcher16_pairs():
            npairs = len(los)
            touched = sorted(los + [l + k for l in los])
            idle = [i for i in range(16) if i not in touched]
            C = bufs[cur]
            lo_dims = [(1024, 2)] + _vdims(los)
            lo_src = view(C[:], los[0] * 64, lo_dims)
            hi_src = view(C[:], (los[0] + k) * 64, lo_dims)
            if len(idle) < npairs:          # ping-pong round
                N = bufs[1 - cur]
                nc.vector.tensor_tensor(view(N[:], los[0] * 64, lo_dims),
                                        lo_src, hi_src, OP.min)
                nc.vector.tensor_tensor(view(N[:], (los[0] + k) * 64, lo_dims),
                                        lo_src, hi_src, OP.max)
                if idle:
                    # off the DVE: Act copies the untouched lanes in parallel
                    idims = [(1024, 2)] + _vdims(idle)
                    nc.scalar.activation(view(N[:], idle[0] * 64, idims),
                                         view(C[:], idle[0] * 64, idims),
                                         mybir.ActivationFunctionType.Copy)
                cur = 1 - cur
            else:                            # in-place round
                dd = _decompose(los)
                tdims = [(npairs * 64, 2)] + (
                    [[dd[1][1] * 64, dd[0][1]], [64, dd[1][1]]]
                    if len(dd) == 2 else [[64, dd[0][1]]]
                ) + [(1, 64)]
                tmp_ap = view(tmp16[:], 0, tdims)
                nc.vector.tensor_tensor(tmp_ap, lo_src, hi_src, OP.max)
                nc.vector.tensor_tensor(lo_src, lo_src, hi_src, OP.min)
                nc.vector.tensor_copy(hi_src, tmp_ap)
        S = bufs[cur]                        # sorted, fp16, (s, v16, hl)

        # ---------------- area_p + area_g on Act: odd-rank sum minus
        # even-rank sum (overlaps with the merge running on the DVE);
        # separate accum tiles avoid false write-serialization
        scr1 = pool.tile([128, 2048], F16, tag="scr1")
        ar1o = pool.tile([128, 1], F32, tag="ar1o")
        ar1e = pool.tile([128, 1], F32, tag="ar1e")
        ar2o = pool.tile([128, 1], F32, tag="ar2o")
        ar2e = pool.tile([128, 1], F32, tag="ar2e")
        ODD = [(128, 16), (1, 64)]                    # every 2nd sv-lane
        nc.scalar.activation(view(scr1[:], 64, ODD), view(S[:], 64, ODD),
                             mybir.ActivationFunctionType.Copy,
                             accum_out=ar1o[:, 0:1])
        nc.scalar.activation(view(scr1[:], 0, ODD), view(S[:], 0, ODD),
                             mybir.ActivationFunctionType.Copy,
                             accum_out=ar1e[:, 0:1])

        # ---------------- merge-32: Batcher odd-even merge (65 comparators,
        # 5 rounds).  The sorted tile S is already [pred asc ++ gt asc] in
        # flat sv-lane order, so round 0 reads S directly -- no init copies.
        # rounds: (distance, lo-indices, idle-indices); None idle = in-place
        M0 = pool.tile([128, 2048], F16, tag="M0")   # (v32, hl)
        M1 = pool.tile([128, 2048], F16, tag="M1")
        OEM = [
            (16, list(range(16)), []),
            (8, list(range(8, 16)), None),            # in-place round
            (4, [4, 5, 6, 7, 12, 13, 14, 15, 20, 21, 22, 23],
             [0, 1, 2, 3, 28, 29, 30, 31]),
            (2, [2, 3, 6, 7, 10, 11, 14, 15, 18, 19, 22, 23, 26, 27],
             [0, 1, 30, 31]),
            (1, list(range(1, 30, 2)), [0, 31]),
        ]
        mbufs = [M0, M1]
        C, mcur = S, 0
        for d, los, idle in OEM:
            ldims = _vdims(los)
            lo_src = view(C[:], los[0] * 64, ldims)
            hi_src = view(C[:], (los[0] + d) * 64, ldims)
            if idle is None:                 # in-place (C is a merge buf)
                tmp_ap = view(tmp16[:], 0,
                              [[64, len(los)], [1, 64]])
                nc.vector.tensor_tensor(tmp_ap, lo_src, hi_src, OP.max)
                nc.vector.tensor_tensor(lo_src, lo_src, hi_src, OP.min)
                nc.vector.tensor_copy(hi_src, tmp_ap)
            else:
                N = mbufs[mcur]
                nc.vector.tensor_tensor(view(N[:], los[0] * 64, ldims),
                                        lo_src, hi_src, OP.min)
                nc.vector.tensor_tensor(view(N[:], (los[0] + d) * 64, ldims),
                                        lo_src, hi_src, OP.max)
                if idle:
                    idims = _vdims(idle)
                    nc.scalar.activation(view(N[:], idle[0] * 64, idims),
                                         view(C[:], idle[0] * 64, idims),
                                         mybir.ActivationFunctionType.Copy)
                C = N
                mcur = 1 - mcur
        M = C

        # ---------------- area_xor tail: the merge preserves the value
        # multiset, so sum(M) == sum(S) == r1o + r1e and
        #   ax = odd(M) - even(M) = 2*odd(M) - (r1o + r1e).
        # Only the odd-rank Act accumulation touches M; the DVE tail is
        # two tiny combines.  Column 0 of the result DMAs out early.
        ar = pool.tile([128, 2], F32, tag="ar")
        nc.vector.tensor_tensor(ar[:, 0:1], ar1o[:, 0:1], ar1e[:, 0:1],
                                OP.subtract)
        nc.sync.dma_start(out=bass.AP(areas_dram.tensor, areas_dram.offset,
                                      [[2, 128], [1, 1]]),
                          in_=ar[:, 0:1])

        scr2 = pool.tile([128, 1024], F16, tag="scr2")
        MODD = [(128, 16), (1, 64)]
        nc.scalar.activation(view(scr2[:], 0, [(64, 16), (1, 64)]),
                             view(M[:], 64, MODD),
                             mybir.ActivationFunctionType.Copy,
                             accum_out=ar2o[:, 0:1])
        s12 = pool.tile([128, 1], F32, tag="s12")
        nc.vector.tensor_tensor(s12[:, 0:1], ar1o[:, 0:1], ar1e[:, 0:1],
                                OP.add)
        nc.vector.scalar_tensor_tensor(ar[:, 1:2], ar2o[:, 0:1], 2.0,
                                       s12[:, 0:1], OP.mult, OP.subtract)
        nc.sync.dma_start(out=bass.AP(areas_dram.tensor, areas_dram.offset + 1,
                                      [[2, 128], [1, 1]]),
                          in_=ar[:, 1:2])


_CACHED_NC = None


def _get_nc():
    global _CACHED_NC
    if _CACHED_NC is not None:
        return _CACHED_NC
    nc = bacc.Bacc("TRN2", target_bir_lowering=False, debug=False,
                   num_devices=N_CORES)
    # declaration order = input staging order: small tensors first so the
    # gather's index tile and the gt vertices land before the 2MB table
    pidx = nc.dram_tensor("pidx", [128, 1], I32, kind="ExternalInput")
    gverts = nc.dram_tensor("gverts", [128, 32], F32, kind="ExternalInput")
    table = nc.dram_tensor("table", [16384, 32], F32, kind="ExternalInput")
    areas = nc.dram_tensor("areas", [128, 2], F32, kind="ExternalOutput")
    with tile.TileContext(nc) as tc:
        _build_core_kernel(tc, areas.ap(), table.ap(), pidx.ap(), gverts.ap())
    nc.compile()
    _CACHED_NC = nc
    return nc


def kernel(output, mask, ind, target):
    global LAST_RESULTS
    output = np.asarray(output)
    mask = np.asarray(mask)
    ind = np.asarray(ind)
    target = np.asarray(target)
    B, C, H, W = output.shape

    # ---- host-side sharding (layout-only)
    in_maps = []
    for c in range(N_CORES):
        b, k0 = c // 2, 64 * (c % 2)
        table = np.ascontiguousarray(output[b].reshape(C, H * W).T).astype(F)
        idx64 = ind[b, k0:k0 + 64].astype(np.int32)
        pidx = np.tile(idx64, 2).reshape(128, 1)
        gv64 = np.ascontiguousarray(target[b, :, k0:k0 + 64].T).astype(F)
        gverts = np.tile(gv64, (2, 1))
        in_maps.append({"table": table, "pidx": pidx, "gverts": gverts})

    nc = _get_nc()
    res = run_bass_kernel_spmd(nc, in_maps, core_ids=list(range(N_CORES)))
    LAST_RESULTS = res

    # ---- host-side gather + final scalar assembly
    spg = np.zeros((B, 128), np.float32)     # area_p + area_g per poly
    ax = np.zeros((B, 128), np.float32)      # area_xor per poly
    for c in range(N_CORES):
        b, k0 = c // 2, 64 * (c % 2)
        halves = res.results[c]["areas"]     # [128, 2]
        spg[b, k0:k0 + 64] = halves[:64, 0] + halves[64:, 0]
        ax[b, k0:k0 + 64] = halves[:64, 1] + halves[64:, 1]
    inter = ((spg - ax) / 2).astype(F)
    union = ((spg + ax) / 2).astype(F)
    iou = (inter / (union + F(1e-4))).astype(F)
    mm = mask.astype(F)
    loss = F(F(1.0) - (iou * mm).sum(dtype=F) / (mm.sum(dtype=F) + F(1e-4)))
    return np.asarray(loss, dtype=np.float32)


# revision 30
# speedup vs baseline: 1.0527x; 1.0527x over previous
"""Trainium2 Bass kernel for the IoU polygon loss (nn_IoUPolyLoss).

Full inputs in, full (scalar) output out. Internally shards the 512
polygons over 8 NeuronCores (64 each: core c -> batch c//2, k-range
64*(c%2)). Rasterization-free: per polygon and scanline, coverage is
the alternating sum of the sorted edge-crossing x-coordinates; the
host combines per-polygon areas into the final loss:
    inter = (area_p + area_g - area_xor) / 2
    union = (area_p + area_g + area_xor) / 2

Device layout per core: partition p = hh*64 + poly (h-half hh), free
dims (side s=2, edge v=16, hl=64); scanline h = hh*64 + hl.

v2 design notes (cost-model driven):
 - affine crossing form xint = py*A + B with per-edge A = dx/dy,
   B = x1 - y1*A (tiny precompute) -> 4 big DVE ops instead of 7
 - crossing mask on the Pool engine in parallel with the DVE chain,
   using bg = roll(ag) (y2 of edge v is y1 of edge v+1)
 - values shifted by -128 before sorting: masked slots (0.0) sort
   ABOVE all real crossings (which are negative) and cancel pairwise
   under the alternating signs, so no sentinel fixup is needed
 - sort-16 / merge-32 in fp16 (integers < 2048 are exact): 2x DVE
   throughput on min/max, 4x on copies; full-coverage rounds are
   ping-ponged (2 ops), sparse rounds run in place (3 ops)
 - final alternating sums via scalar_tensor_tensor accum_out: one
   instruction per area (host only ever needs ap+ag and ax)
"""
import sys

import numpy as np

try:
    import concourse.bass as bass
except ImportError:
    sys.path.insert(0, "/opt/trn_rl_repo")
    import concourse.bass as bass

import concourse.mybir as mybir
import concourse.tile as tile
import concourse.bacc as bacc
from concourse.bass_utils import run_bass_kernel_spmd

OP = mybir.AluOpType
F32 = mybir.dt.float32
F16 = mybir.dt.float16
I32 = mybir.dt.int32
F = np.float32

MAGIC = 12582912.0            # 1.5 * 2^23, RN-to-int trick for |x| < 2^22
KK = 0.49545454545454547      # 0.5 - 1/220 margin
M2 = MAGIC + 128.0            # exact in fp32
MAGICM100 = MAGIC - 100.0     # exact in fp32

N_CORES = 8

LAST_RESULTS = None           # BassKernelResults of the most recent run


def _batcher16_pairs():
    n = 16
    rounds = []
    p = 1
    while p < n:
        k = p
        while k >= 1:
            los = []
            j = k % p
            while j <= n - 1 - k:
                for i in range(0, min(k, n - j - k)):
                    if (i + j) // (2 * p) == (i + j + k) // (2 * p):
                        los.append(i + j)
                j += 2 * k
            rounds.append((k, los))
            k //= 2
        p *= 2
    return rounds


def _decompose(idxs):
    n = len(idxs)
    if n == 1:
        return [[1, 1]]
    d = idxs[1] - idxs[0]
    if all(idxs[i] == idxs[0] + i * d for i in range(n)):
        return [[d, n]]
    run = 1
    while run < n and idxs[run] == idxs[0] + run * d:
        run += 1
    assert n % run == 0, f"cannot decompose {idxs}"
    outer = idxs[::run]
    do = outer[1] - outer[0]
    for oi, o in enumerate(outer):
        assert o == outer[0] + oi * do
        for ii in range(run):
            assert idxs[oi * run + ii] == o + ii * d, f"cannot decompose {idxs}"
    return [[do, len(outer)], [d, run]]


def _view(tile_ap, offset, dims):
    return bass.AP(
        tile_ap.tensor,
        tile_ap.offset + offset,
        [list(tile_ap.ap[0])] + [[s, c] for s, c in dims],
    )


def _vdims(idxs, inner=64):
    """AP dims for a set of v-indices (times stride 64, hl inner)."""
    return [[s * 64, c] for s, c in _decompose(idxs)] + [[1, inner]]


def _build_core_kernel(tc, areas_dram, table, pidx, gverts):
    nc = tc.nc
    view = _view
    with tc.tile_pool(name="main", bufs=1) as pool:
        # ---------------- loads: separate tiles so the gt DMA is not
        # serialized behind the (conservatively-tracked) indirect gather
        pidx_sb = pool.tile([128, 1], I32, tag="pidx")
        nc.sync.dma_start(out=pidx_sb[:], in_=pidx)
        rawp = pool.tile([128, 32], F32, tag="rawp")
        nc.gpsimd.indirect_dma_start(
            out=rawp[:],
            out_offset=None,
            in_=table,
            in_offset=bass.IndirectOffsetOnAxis(ap=pidx_sb[:, :1], axis=0),
        )
        rawg = pool.tile([128, 32], F32, tag="rawg")
        nc.sync.dma_start(out=rawg[:], in_=gverts)

        # ---------------- py = hh*64 + hl  (f32 [128, 64]; needs only iotas)
        hlq = pool.tile([128, 64], I32, tag="hlq")
        nc.gpsimd.iota(hlq[:], pattern=[[1, 64]], base=0, channel_multiplier=0)
        pid = pool.tile([128, 1], I32, tag="pid")
        nc.gpsimd.iota(pid[:], pattern=[[0, 1]], base=0, channel_multiplier=1)
        hh64 = pool.tile([128, 1], I32, tag="hh64")
        nc.vector.tensor_scalar(hh64[:], pid[:], 64, None, OP.bitwise_and)
        hh64f = pool.tile([128, 1], F32, tag="hh64f")
        nc.vector.tensor_copy(hh64f[:], hh64[:])
        hlf = pool.tile([128, 64], F32, tag="hlf")
        nc.vector.tensor_copy(hlf[:], hlq[:])
        pyf = pool.tile([128, 64], F32, tag="pyf")
        nc.vector.tensor_scalar(pyf[:], hlf[:], hh64f[:, :1], None, OP.add)
        pyh = pool.tile([128, 64], F16, tag="pyh")
        nc.vector.tensor_copy(pyh[:], pyf[:])

        # ---------------- per-side pipeline: trunc -> edges -> crossing
        # mask.  The gt side (direct DMA) runs entirely inside the window
        # where the DVE would otherwise idle waiting for the pred gather.
        pk = [(16, 2), (1, 16)]          # packed (s, v) dims for [128,32]
        verts = pool.tile([128, 64], F32, tag="verts")   # (s, v, coord)
        ver2 = pool.tile([128, 64], F32, tag="ver2")     # roll v by 1
        dall = pool.tile([128, 64], F32, tag="dall")     # (dx, dy) per edge
        z = pool.tile([128, 32], F32, tag="z")
        ds = pool.tile([128, 32], F32, tag="ds")
        rt = pool.tile([128, 32], F32, tag="rt")
        At = pool.tile([128, 32], F32, tag="At")
        n1 = pool.tile([128, 32], F32, tag="n1")
        Bt = pool.tile([128, 32], F32, tag="Bt")
        y1h = pool.tile([128, 32], F16, tag="y1h")
        agt = pool.tile([128, 2048], F16, tag="ag")
        crt = pool.tile([128, 2048], F16, tag="cr")

        for off, src in ((32, rawg), (0, rawp)):
            sv = off // 2                # v-column base within (s,v) packing
            # trunc(x)+100 via round(x - 0.5*sgn(x)) + 100
            m = pool.tile([128, 32], F32, tag=f"m{off}")
            nc.vector.tensor_scalar(m[:], src[:], 0.0, None, OP.is_ge)
            sh = pool.tile([128, 32], F32, tag=f"sh{off}")
            nc.vector.tensor_scalar(sh[:], m[:], -1.0, 0.5, OP.mult, OP.add)
            u = pool.tile([128, 32], F32, tag=f"u{off}")
            nc.vector.tensor_tensor(u[:], src[:], sh[:], OP.add)
            nc.vector.tensor_scalar(verts[:, off:off + 32], u[:],
                                    MAGIC, MAGICM100, OP.add, OP.subtract)
            # edge precompute: A = dx/dy', B = x1 - y1*A
            nc.vector.tensor_copy(view(ver2[:], off, [(2, 15), (1, 2)]),
                                  view(verts[:], off + 2, [(2, 15), (1, 2)]))
            nc.vector.tensor_copy(view(ver2[:], off + 30, [(1, 2)]),
                                  view(verts[:], off, [(1, 2)]))
            nc.vector.tensor_tensor(dall[:, off:off + 32],
                                    ver2[:, off:off + 32],
                                    verts[:, off:off + 32], OP.subtract)
            pv = [(1, 16)]
            x1v = view(verts[:], off, [(2, 16)])
            y1v = view(verts[:], off + 1, [(2, 16)])
            dxv = view(dall[:], off, [(2, 16)])
            dyv = view(dall[:], off + 1, [(2, 16)])
            nc.vector.tensor_scalar(view(z[:], sv, pv), dyv, 0.0, None,
                                    OP.is_equal)
            nc.vector.tensor_tensor(view(ds[:], sv, pv), dyv,
                                    view(z[:], sv, pv), OP.add)
            nc.vector.reciprocal(rt[:, sv:sv + 16], ds[:, sv:sv + 16])
            nc.vector.tensor_tensor(view(At[:], sv, pv),
                                    view(rt[:], sv, pv), dxv, OP.mult)
            nc.vector.tensor_tensor(view(n1[:], sv, pv), y1v,
                                    view(At[:], sv, pv), OP.mult)
            nc.vector.scalar_tensor_tensor(view(Bt[:], sv, pv),
                                           view(n1[:], sv, pv), -1.0, x1v,
                                           OP.mult, OP.add)
            nc.vector.tensor_copy(view(y1h[:], sv, pv), y1v)
            # crossing mask: ag = (y1 <= py); crg = ag != roll(ag)
            so = sv * 64                 # sv-lane base in the fp16 grid
            nc.vector.tensor_tensor(view(agt[:], so, [(64, 16), (1, 64)]),
                                    view(y1h[:], sv, [(1, 16), (0, 64)]),
                                    view(pyh[:], 0, [(0, 16), (1, 64)]),
                                    OP.is_le)
            nc.vector.tensor_tensor(
                view(crt[:], so, [(64, 15), (1, 64)]),
                view(agt[:], so + 64, [(64, 15), (1, 64)]),
                view(agt[:], so, [(64, 15), (1, 64)]), OP.not_equal)
            nc.vector.tensor_tensor(
                view(crt[:], so + 15 * 64, [(1, 64)]),
                view(agt[:], so, [(1, 64)]),
                view(agt[:], so + 15 * 64, [(1, 64)]), OP.not_equal)

        # ---------------- grid stage, free = (s2, v16, hl64) = 2048
        GD = [(1024, 2), (64, 16), (1, 64)]
        Ag = view(At[:], 0, [(16, 2), (1, 16), (0, 64)])
        pyg = view(pyf[:], 0, [(0, 2), (0, 16), (1, 64)])

        def gtile(tag, dt=F32):
            t = pool.tile([128, 2048], dt, tag=tag)
            return t, view(t[:], 0, GD)

        # flat (s*v, hl) views: scalar_tensor_tensor needs <=2 free dims
        GF = [(64, 32), (1, 64)]
        BF = [(1, 32), (0, 64)]              # per-edge smalls broadcast on hl

        # xint chain split into hl-halves so the Act-engine rounding of
        # half A overlaps the DVE computing half B (no DVE idle bubble):
        #   DVE: t1 = py*A ; xkk = (t1+KK)+B     Act: wr = xkk+MAGIC
        #   DVE: cg = (wr - (MAGIC+128)) * crg  -> fp16, shifted by -128
        t1, t1g = gtile("t1")
        xkk, xkkg = gtile("xkk")
        wr, wrg = gtile("wr")
        T0, T0g = gtile("T0", F16)
        T1, T1g = gtile("T1", F16)
        GH = [(64, 32), (1, 32)]             # one hl-half, flat (s*v, hl)
        for h in (0, 32):
            nc.vector.tensor_tensor(
                view(t1[:], h, [(1024, 2), (64, 16), (1, 32)]),
                view(pyf[:], h, [(0, 2), (0, 16), (1, 32)]),
                view(At[:], 0, [(16, 2), (1, 16), (0, 32)]), OP.mult)
            nc.vector.scalar_tensor_tensor(
                view(xkk[:], h, GH), view(t1[:], h, GH),
                KK, view(Bt[:], 0, [(1, 32), (0, 32)]), OP.add, OP.add)
            nc.scalar.activation(view(wr[:], h, GH), view(xkk[:], h, GH),
                                 mybir.ActivationFunctionType.Copy, bias=MAGIC)
        for h in (0, 32):
            nc.vector.scalar_tensor_tensor(view(T0[:], h, GH),
                                           view(wr[:], h, GH),
                                           M2, view(crt[:], h, GH),
                                           OP.subtract, OP.mult)

        # ---------------- sort-16 along v (ascending), fp16
        # ping-pong when idle < np, else in-place; track current buffer
        tmp16 = pool.tile([128, 1024], F16, tag="tmp16")
        bufs = [T0, T1]
        cur = 0
        for k, los in _batcher16_pairs():
            npairs = len(los)
            touched = sorted(los + [l + k for l in los])
            idle = [i for i in range(16) if i not in touched]
            C = bufs[cur]
            lo_dims = [(1024, 2)] + _vdims(los)
            lo_src = view(C[:], los[0] * 64, lo_dims)
            hi_src = view(C[:], (los[0] + k) * 64, lo_dims)
            if len(idle) < npairs:          # ping-pong round
                N = bufs[1 - cur]
                nc.vector.tensor_tensor(view(N[:], los[0] * 64, lo_dims),
                                        lo_src, hi_src, OP.min)
                nc.vector.tensor_tensor(view(N[:], (los[0] + k) * 64, lo_dims),
                                        lo_src, hi_src, OP.max)
                if idle:
                    # off the DVE: Act copies the untouched lanes in parallel
                    idims = [(1024, 2)] + _vdims(idle)
                    nc.scalar.activation(view(N[:], idle[0] * 64, idims),
                                         view(C[:], idle[0] * 64, idims),
                                         mybir.ActivationFunctionType.Copy)
                cur = 1 - cur
            else:                            # in-place round
                dd = _decompose(los)
                tdims = [(npairs * 64, 2)] + (
                    [[dd[1][1] * 64, dd[0][1]], [64, dd[1][1]]]
                    if len(dd) == 2 else [[64, dd[0][1]]]
                ) + [(1, 64)]
                tmp_ap = view(tmp16[:], 0, tdims)
                nc.vector.tensor_tensor(tmp_ap, lo_src, hi_src, OP.max)
                nc.vector.tensor_tensor(lo_src, lo_src, hi_src, OP.min)
                nc.vector.tensor_copy(hi_src, tmp_ap)
        S = bufs[cur]                        # sorted, fp16, (s, v16, hl)

        # ---------------- area_p + area_g on Act: odd-rank sum minus
        # even-rank sum (overlaps with the merge running on the DVE);
        # separate accum tiles avoid false write-serialization
        scr1 = pool.tile([128, 2048], F16, tag="scr1")
        ar1o = pool.tile([128, 1], F32, tag="ar1o")
        ar1e = pool.tile([128, 1], F32, tag="ar1e")
        ar2o = pool.tile([128, 1], F32, tag="ar2o")
        ar2e = pool.tile([128, 1], F32, tag="ar2e")
        ODD = [(128, 16), (1, 64)]                    # every 2nd sv-lane
        nc.scalar.activation(view(scr1[:], 64, ODD), view(S[:], 64, ODD),
                             mybir.ActivationFunctionType.Copy,
                             accum_out=ar1o[:, 0:1])
        nc.scalar.activation(view(scr1[:], 0, ODD), view(S[:], 0, ODD),
                             mybir.ActivationFunctionType.Copy,
                             accum_out=ar1e[:, 0:1])

        # ---------------- merge-32: Batcher odd-even merge (65 comparators,
        # 5 rounds).  The sorted tile S is already [pred asc ++ gt asc] in
        # flat sv-lane order, so round 0 reads S directly -- no init copies.
        # rounds: (distance, lo-indices, idle-indices); None idle = in-place
        M0 = pool.tile([128, 2048], F16, tag="M0")   # (v32, hl)
        M1 = pool.tile([128, 2048], F16, tag="M1")
        OEM = [
            (16, list(range(16)), []),
            (8, list(range(8, 16)), None),            # in-place round
            (4, [4, 5, 6, 7, 12, 13, 14, 15, 20, 21, 22, 23],
             [0, 1, 2, 3, 28, 29, 30, 31]),
            (2, [2, 3, 6, 7, 10, 11, 14, 15, 18, 19, 22, 23, 26, 27],
             [0, 1, 30, 31]),
            (1, list(range(1, 30, 2)), [0, 31]),
        ]
        mbufs = [M0, M1]
        C, mcur = S, 0
        for d, los, idle in OEM:
            ldims = _vdims(los)
            lo_src = view(C[:], los[0] * 64, ldims)
            hi_src = view(C[:], (los[0] + d) * 64, ldims)
            if idle is None:                 # in-place (C is a merge buf)
                tmp_ap = view(tmp16[:], 0,
                              [[64, len(los)], [1, 64]])
                nc.vector.tensor_tensor(tmp_ap, lo_src, hi_src, OP.max)
                nc.vector.tensor_tensor(lo_src, lo_src, hi_src, OP.min)
                nc.vector.tensor_copy(hi_src, tmp_ap)
            else:
                N = mbufs[mcur]
                nc.vector.tensor_tensor(view(N[:], los[0] * 64, ldims),
                                        lo_src, hi_src, OP.min)
                nc.vector.tensor_tensor(view(N[:], (los[0] + d) * 64, ldims),
                                        lo_src, hi_src, OP.max)
                if idle:
                    idims = _vdims(idle)
                    nc.scalar.activation(view(N[:], idle[0] * 64, idims),
                                         view(C[:], idle[0] * 64, idims),
                                         mybir.ActivationFunctionType.Copy)
                C = N
                mcur = 1 - mcur
        M = C

        # ---------------- area_xor tail: the merge preserves the value
        # multiset, so sum(M) == sum(S) == r1o + r1e and
        #   ax = odd(M) - even(M) = 2*odd(M) - (r1o + r1e).
        # Only the odd-rank Act accumulation touches M; the DVE tail is
        # two tiny combines.  Column 0 of the result DMAs out early.
        ar = pool.tile([128, 2], F32, tag="ar")
        nc.vector.tensor_tensor(ar[:, 0:1], ar1o[:, 0:1], ar1e[:, 0:1],
                                OP.subtract)
        nc.sync.dma_start(out=bass.AP(areas_dram.tensor, areas_dram.offset,
                                      [[2, 128], [1, 1]]),
                          in_=ar[:, 0:1])

        scr2 = pool.tile([128, 1024], F16, tag="scr2")
        MODD = [(128, 16), (1, 64)]
        nc.scalar.activation(view(scr2[:], 0, [(64, 16), (1, 64)]),
                             view(M[:], 64, MODD),
                             mybir.ActivationFunctionType.Copy,
                             accum_out=ar2o[:, 0:1])
        s12 = pool.tile([128, 1], F32, tag="s12")
        nc.vector.tensor_tensor(s12[:, 0:1], ar1o[:, 0:1], ar1e[:, 0:1],
                                OP.add)
        nc.vector.scalar_tensor_tensor(ar[:, 1:2], ar2o[:, 0:1], 2.0,
                                       s12[:, 0:1], OP.mult, OP.subtract)
        nc.sync.dma_start(out=bass.AP(areas_dram.tensor, areas_dram.offset + 1,
                                      [[2, 128], [1, 1]]),
                          in_=ar[:, 1:2])


_CACHED_NC = None


def _get_nc():
    global _CACHED_NC
    if _CACHED_NC is not None:
        return _CACHED_NC
    nc = bacc.Bacc("TRN2", target_bir_lowering=False, debug=False,
                   num_devices=N_CORES)
    # declaration order = input staging order: small tensors first so the
    # gather's index tile and the gt vertices land before the 2MB table
    pidx = nc.dram_tensor("pidx", [128, 1], I32, kind="ExternalInput")
    gverts = nc.dram_tensor("gverts", [128, 32], F32, kind="ExternalInput")
    table = nc.dram_tensor("table", [16384, 32], F32, kind="ExternalInput")
    areas = nc.dram_tensor("areas", [128, 2], F32, kind="ExternalOutput")
    with tile.TileContext(nc) as tc:
        _build_core_kernel(tc, areas.ap(), table.ap(), pidx.ap(), gverts.ap())
    nc.compile()
    _CACHED_NC = nc
    return nc


def kernel(output, mask, ind, target):
    global LAST_RESULTS
    output = np.asarray(output)
    mask = np.asarray(mask)
    ind = np.asarray(ind)
    target = np.asarray(target)
    B, C, H, W = output.shape

    # ---- host-side sharding (layout-only)
    in_maps = []
    for c in range(N_CORES):
        b, k0 = c // 2, 64 * (c % 2)
        table = np.ascontiguousarray(output[b].reshape(C, H * W).T).astype(F)
        idx64 = ind[b, k0:k0 + 64].astype(np.int32)
        pidx = np.tile(idx64, 2).reshape(128, 1)
        gv64 = np.ascontiguousarray(target[b, :, k0:k0 + 64].T).astype(F)
        gverts = np.tile(gv64, (2, 1))
        in_maps.append({"table": table, "pidx": pidx, "gverts": gverts})

    nc = _get_nc()
    res = run_bass_kernel_spmd(nc, in_maps, core_ids=list(range(N_CORES)))
    LAST_RESULTS = res

    # ---- host-side gather + final scalar assembly
    spg = np.zeros((B, 128), np.float32)     # area_p + area_g per poly
    ax = np.zeros((B, 128), np.float32)      # area_xor per poly
    for c in range(N_CORES):
        b, k0 = c // 2, 64 * (c % 2)
        halves = res.results[c]["areas"]     # [128, 2]
        spg[b, k0:k0 + 64] = halves[:64, 0] + halves[64:, 0]
        ax[b, k0:k0 + 64] = halves[:64, 1] + halves[64:, 1]
    inter = ((spg - ax) / 2).astype(F)
    union = ((spg + ax) / 2).astype(F)
    iou = (inter / (union + F(1e-4))).astype(F)
    mm = mask.astype(F)
    loss = F(F(1.0) - (iou * mm).sum(dtype=F) / (mm.sum(dtype=F) + F(1e-4)))
    return np.asarray(loss, dtype=np.float32)


# revision 31
# speedup vs baseline: 1.0735x; 1.0198x over previous
"""Trainium2 Bass kernel for the IoU polygon loss (nn_IoUPolyLoss).

Full inputs in, full (scalar) output out. Internally shards the 512
polygons over 8 NeuronCores (64 each: core c -> batch c//2, k-range
64*(c%2)). Rasterization-free: per polygon and scanline, coverage is
the alternating sum of the sorted edge-crossing x-coordinates; the
host combines per-polygon areas into the final loss:
    inter = (area_p + area_g - area_xor) / 2
    union = (area_p + area_g + area_xor) / 2

Device layout per core: partition p = hh*64 + poly (h-half hh), free
dims (side s=2, edge v=16, hl=64); scanline h = hh*64 + hl.

v2 design notes (cost-model driven):
 - affine crossing form xint = py*A + B with per-edge A = dx/dy,
   B = x1 - y1*A (tiny precompute) -> 4 big DVE ops instead of 7
 - crossing mask on the Pool engine in parallel with the DVE chain,
   using bg = roll(ag) (y2 of edge v is y1 of edge v+1)
 - values shifted by -128 before sorting: masked slots (0.0) sort
   ABOVE all real crossings (which are negative) and cancel pairwise
   under the alternating signs, so no sentinel fixup is needed
 - sort-16 / merge-32 in fp16 (integers < 2048 are exact): 2x DVE
   throughput on min/max, 4x on copies; full-coverage rounds are
   ping-ponged (2 ops), sparse rounds run in place (3 ops)
 - final alternating sums via scalar_tensor_tensor accum_out: one
   instruction per area (host only ever needs ap+ag and ax)
"""
import sys

import numpy as np

try:
    import concourse.bass as bass
except ImportError:
    sys.path.insert(0, "/opt/trn_rl_repo")
    import concourse.bass as bass

import concourse.mybir as mybir
import concourse.tile as tile
import concourse.bacc as bacc
from concourse.bass_utils import run_bass_kernel_spmd

OP = mybir.AluOpType
F32 = mybir.dt.float32
F16 = mybir.dt.float16
I32 = mybir.dt.int32
F = np.float32

MAGIC = 12582912.0            # 1.5 * 2^23, RN-to-int trick for |x| < 2^22
KK = 0.49545454545454547      # 0.5 - 1/220 margin
M2 = MAGIC + 128.0            # exact in fp32
MAGICM100 = MAGIC - 100.0     # exact in fp32

N_CORES = 8

LAST_RESULTS = None           # BassKernelResults of the most recent run


def _batcher16_pairs():
    n = 16
    rounds = []
    p = 1
    while p < n:
        k = p
        while k >= 1:
            los = []
            j = k % p
            while j <= n - 1 - k:
                for i in range(0, min(k, n - j - k)):
                    if (i + j) // (2 * p) == (i + j + k) // (2 * p):
                        los.append(i + j)
                j += 2 * k
            rounds.append((k, los))
            k //= 2
        p *= 2
    return rounds


def _decompose(idxs):
    n = len(idxs)
    if n == 1:
        return [[1, 1]]
    d = idxs[1] - idxs[0]
    if all(idxs[i] == idxs[0] + i * d for i in range(n)):
        return [[d, n]]
    run = 1
    while run < n and idxs[run] == idxs[0] + run * d:
        run += 1
    assert n % run == 0, f"cannot decompose {idxs}"
    outer = idxs[::run]
    do = outer[1] - outer[0]
    for oi, o in enumerate(outer):
        assert o == outer[0] + oi * do
        for ii in range(run):
            assert idxs[oi * run + ii] == o + ii * d, f"cannot decompose {idxs}"
    return [[do, len(outer)], [d, run]]


def _view(tile_ap, offset, dims):
    return bass.AP(
        tile_ap.tensor,
        tile_ap.offset + offset,
        [list(tile_ap.ap[0])] + [[s, c] for s, c in dims],
    )


def _vdims(idxs, inner=64):
    """AP dims for a set of v-indices (times stride 64, hl inner)."""
    return [[s * 64, c] for s, c in _decompose(idxs)] + [[1, inner]]


def _build_core_kernel(tc, areas_dram, table, pidx, gverts):
    nc = tc.nc
    view = _view
    with tc.tile_pool(name="main", bufs=1) as pool:
        # ---------------- loads: separate tiles so the gt DMA is not
        # serialized behind the (conservatively-tracked) indirect gather
        pidx_sb = pool.tile([128, 1], I32, tag="pidx")
        nc.sync.dma_start(out=pidx_sb[:], in_=pidx)
        rawp = pool.tile([128, 32], F32, tag="rawp")
        nc.gpsimd.indirect_dma_start(
            out=rawp[:],
            out_offset=None,
            in_=table,
            in_offset=bass.IndirectOffsetOnAxis(ap=pidx_sb[:, :1], axis=0),
        )
        rawg = pool.tile([128, 32], F32, tag="rawg")
        nc.sync.dma_start(out=rawg[:], in_=gverts)

        # ---------------- py = hh*64 + hl  (f32 [128, 64]; needs only iotas)
        hlq = pool.tile([128, 64], I32, tag="hlq")
        nc.gpsimd.iota(hlq[:], pattern=[[1, 64]], base=0, channel_multiplier=0)
        pid = pool.tile([128, 1], I32, tag="pid")
        nc.gpsimd.iota(pid[:], pattern=[[0, 1]], base=0, channel_multiplier=1)
        hh64 = pool.tile([128, 1], I32, tag="hh64")
        nc.vector.tensor_scalar(hh64[:], pid[:], 64, None, OP.bitwise_and)
        hh64f = pool.tile([128, 1], F32, tag="hh64f")
        nc.vector.tensor_copy(hh64f[:], hh64[:])
        hlf = pool.tile([128, 64], F32, tag="hlf")
        nc.vector.tensor_copy(hlf[:], hlq[:])
        pyf = pool.tile([128, 64], F32, tag="pyf")
        nc.vector.tensor_scalar(pyf[:], hlf[:], hh64f[:, :1], None, OP.add)
        pyh = pool.tile([128, 64], F16, tag="pyh")
        nc.vector.tensor_copy(pyh[:], pyf[:])

        # ---------------- per-side pipeline: trunc -> edges -> crossing
        # mask.  The gt side (direct DMA) runs entirely inside the window
        # where the DVE would otherwise idle waiting for the pred gather.
        pk = [(16, 2), (1, 16)]          # packed (s, v) dims for [128,32]
        verts = pool.tile([128, 64], F32, tag="verts")   # (s, v, coord)
        ver2 = pool.tile([128, 64], F32, tag="ver2")     # roll v by 1
        dall = pool.tile([128, 64], F32, tag="dall")     # (dx, dy) per edge
        z = pool.tile([128, 32], F32, tag="z")
        ds = pool.tile([128, 32], F32, tag="ds")
        rt = pool.tile([128, 32], F32, tag="rt")
        At = pool.tile([128, 32], F32, tag="At")
        n1 = pool.tile([128, 32], F32, tag="n1")
        Bt = pool.tile([128, 32], F32, tag="Bt")
        y1h = pool.tile([128, 32], F16, tag="y1h")
        agt = pool.tile([128, 2048], F16, tag="ag")
        crt = pool.tile([128, 2048], F16, tag="cr")

        for off, src in ((32, rawg), (0, rawp)):
            sv = off // 2                # v-column base within (s,v) packing
            # trunc(x)+100 via round(x - 0.5*sgn(x)) + 100
            m = pool.tile([128, 32], F32, tag=f"m{off}")
            nc.vector.tensor_scalar(m[:], src[:], 0.0, None, OP.is_ge)
            sh = pool.tile([128, 32], F32, tag=f"sh{off}")
            nc.vector.tensor_scalar(sh[:], m[:], -1.0, 0.5, OP.mult, OP.add)
            u = pool.tile([128, 32], F32, tag=f"u{off}")
            nc.vector.tensor_tensor(u[:], src[:], sh[:], OP.add)
            nc.vector.tensor_scalar(verts[:, off:off + 32], u[:],
                                    MAGIC, MAGICM100, OP.add, OP.subtract)
            # edge precompute: A = dx/dy', B = x1 - y1*A
            nc.vector.tensor_copy(view(ver2[:], off, [(2, 15), (1, 2)]),
                                  view(verts[:], off + 2, [(2, 15), (1, 2)]))
            nc.vector.tensor_copy(view(ver2[:], off + 30, [(1, 2)]),
                                  view(verts[:], off, [(1, 2)]))
            nc.vector.tensor_tensor(dall[:, off:off + 32],
                                    ver2[:, off:off + 32],
                                    verts[:, off:off + 32], OP.subtract)
            pv = [(1, 16)]
            x1v = view(verts[:], off, [(2, 16)])
            y1v = view(verts[:], off + 1, [(2, 16)])
            dxv = view(dall[:], off, [(2, 16)])
            dyv = view(dall[:], off + 1, [(2, 16)])
            nc.vector.tensor_scalar(view(z[:], sv, pv), dyv, 0.0, None,
                                    OP.is_equal)
            nc.vector.tensor_tensor(view(ds[:], sv, pv), dyv,
                                    view(z[:], sv, pv), OP.add)
            nc.vector.reciprocal(rt[:, sv:sv + 16], ds[:, sv:sv + 16])
            nc.vector.tensor_tensor(view(At[:], sv, pv),
                                    view(rt[:], sv, pv), dxv, OP.mult)
            nc.vector.tensor_tensor(view(n1[:], sv, pv), y1v,
                                    view(At[:], sv, pv), OP.mult)
            nc.vector.scalar_tensor_tensor(view(Bt[:], sv, pv),
                                           view(n1[:], sv, pv), -1.0, x1v,
                                           OP.mult, OP.add)
            nc.vector.tensor_copy(view(y1h[:], sv, pv), y1v)
            # crossing mask: ag = (y1 <= py); crg = ag != roll(ag)
            so = sv * 64                 # sv-lane base in the fp16 grid
            nc.vector.tensor_tensor(view(agt[:], so, [(64, 16), (1, 64)]),
                                    view(y1h[:], sv, [(1, 16), (0, 64)]),
                                    view(pyh[:], 0, [(0, 16), (1, 64)]),
                                    OP.is_le)
            nc.vector.tensor_tensor(
                view(crt[:], so, [(64, 15), (1, 64)]),
                view(agt[:], so + 64, [(64, 15), (1, 64)]),
                view(agt[:], so, [(64, 15), (1, 64)]), OP.not_equal)
            nc.vector.tensor_tensor(
                view(crt[:], so + 15 * 64, [(1, 64)]),
                view(agt[:], so, [(1, 64)]),
                view(agt[:], so + 15 * 64, [(1, 64)]), OP.not_equal)

        # ---------------- grid stage, free = (s2, v16, hl64) = 2048
        GD = [(1024, 2), (64, 16), (1, 64)]
        Ag = view(At[:], 0, [(16, 2), (1, 16), (0, 64)])
        pyg = view(pyf[:], 0, [(0, 2), (0, 16), (1, 64)])

        def gtile(tag, dt=F32):
            t = pool.tile([128, 2048], dt, tag=tag)
            return t, view(t[:], 0, GD)

        # flat (s*v, hl) views: scalar_tensor_tensor needs <=2 free dims
        GF = [(64, 32), (1, 64)]
        BF = [(1, 32), (0, 64)]              # per-edge smalls broadcast on hl

        # xint chain split into hl-halves so the Act-engine rounding of
        # half A overlaps the DVE computing half B (no DVE idle bubble):
        #   DVE: t1 = py*A ; xkk = (t1+KK)+B     Act: wr = xkk+MAGIC
        #   DVE: cg = (wr - (MAGIC+128)) * crg  -> fp16, shifted by -128
        t1, t1g = gtile("t1")
        xkk, xkkg = gtile("xkk")
        wr, wrg = gtile("wr")
        T0, T0g = gtile("T0", F16)
        T1, T1g = gtile("T1", F16)
        GH = [(64, 32), (1, 32)]             # one hl-half, flat (s*v, hl)
        for h in (0, 32):
            nc.vector.tensor_tensor(
                view(t1[:], h, [(1024, 2), (64, 16), (1, 32)]),
                view(pyf[:], h, [(0, 2), (0, 16), (1, 32)]),
                view(At[:], 0, [(16, 2), (1, 16), (0, 32)]), OP.mult)
            nc.vector.scalar_tensor_tensor(
                view(xkk[:], h, GH), view(t1[:], h, GH),
                KK, view(Bt[:], 0, [(1, 32), (0, 32)]), OP.add, OP.add)
            nc.scalar.activation(view(wr[:], h, GH), view(xkk[:], h, GH),
                                 mybir.ActivationFunctionType.Copy, bias=MAGIC)
        for h in (0, 32):
            nc.vector.scalar_tensor_tensor(view(T0[:], h, GH),
                                           view(wr[:], h, GH),
                                           M2, view(crt[:], h, GH),
                                           OP.subtract, OP.mult)

        # ---------------- sort-16 along v (ascending), fp16
        # ping-pong when idle < np, else in-place; track current buffer
        tmp16 = pool.tile([128, 1024], F16, tag="tmp16")
        bufs = [T0, T1]
        cur = 0
        for k, los in _batcher16_pairs():
            npairs = len(los)
            touched = sorted(los + [l + k for l in los])
            idle = [i for i in range(16) if i not in touched]
            C = bufs[cur]
            lo_dims = [(1024, 2)] + _vdims(los)
            lo_src = view(C[:], los[0] * 64, lo_dims)
            hi_src = view(C[:], (los[0] + k) * 64, lo_dims)
            if len(idle) < npairs:          # ping-pong round
                N = bufs[1 - cur]
                nc.vector.tensor_tensor(view(N[:], los[0] * 64, lo_dims),
                                        lo_src, hi_src, OP.min)
                nc.vector.tensor_tensor(view(N[:], (los[0] + k) * 64, lo_dims),
                                        lo_src, hi_src, OP.max)
                if idle:
                    # off the DVE: Act copies the untouched lanes in parallel
                    idims = [(1024, 2)] + _vdims(idle)
                    nc.scalar.activation(view(N[:], idle[0] * 64, idims),
                                         view(C[:], idle[0] * 64, idims),
                                         mybir.ActivationFunctionType.Copy)
                cur = 1 - cur
            else:                            # in-place round
                dd = _decompose(los)
                tdims = [(npairs * 64, 2)] + (
                    [[dd[1][1] * 64, dd[0][1]], [64, dd[1][1]]]
                    if len(dd) == 2 else [[64, dd[0][1]]]
                ) + [(1, 64)]
                tmp_ap = view(tmp16[:], 0, tdims)
                nc.vector.tensor_tensor(tmp_ap, lo_src, hi_src, OP.max)
                nc.vector.tensor_tensor(lo_src, lo_src, hi_src, OP.min)
                nc.vector.tensor_copy(hi_src, tmp_ap)
        S = bufs[cur]                        # sorted, fp16, (s, v16, hl)

        # ---------------- area_p + area_g on Act: odd-rank sum minus
        # even-rank sum (overlaps with the merge running on the DVE);
        # separate accum tiles avoid false write-serialization
        scr1 = pool.tile([128, 2048], F16, tag="scr1")
        ar1o = pool.tile([128, 1], F32, tag="ar1o")
        ar1e = pool.tile([128, 1], F32, tag="ar1e")
        ar2o = pool.tile([128, 1], F32, tag="ar2o")
        ar2e = pool.tile([128, 1], F32, tag="ar2e")
        ODD = [(128, 16), (1, 64)]                    # every 2nd sv-lane
        nc.scalar.activation(view(scr1[:], 64, ODD), view(S[:], 64, ODD),
                             mybir.ActivationFunctionType.Copy,
                             accum_out=ar1o[:, 0:1])
        nc.scalar.activation(view(scr1[:], 0, ODD), view(S[:], 0, ODD),
                             mybir.ActivationFunctionType.Copy,
                             accum_out=ar1e[:, 0:1])

        # ---------------- merge-32: Batcher odd-even merge (65 comparators,
        # 5 rounds).  The sorted tile S is already [pred asc ++ gt asc] in
        # flat sv-lane order, so round 0 reads S directly -- no init copies.
        # rounds: (distance, lo-indices, idle-indices); None idle = in-place
        M0 = pool.tile([128, 2048], F16, tag="M0")   # (v32, hl)
        M1 = pool.tile([128, 2048], F16, tag="M1")
        OEM = [
            (16, list(range(16)), []),
            (8, list(range(8, 16)), None),            # in-place round
            (4, [4, 5, 6, 7, 12, 13, 14, 15, 20, 21, 22, 23],
             [0, 1, 2, 3, 28, 29, 30, 31]),
            (2, [2, 3, 6, 7, 10, 11, 14, 15, 18, 19, 22, 23, 26, 27],
             [0, 1, 30, 31]),
        ]
        mbufs = [M0, M1]
        C, mcur = S, 0
        for d, los, idle in OEM:
            ldims = _vdims(los)
            lo_src = view(C[:], los[0] * 64, ldims)
            hi_src = view(C[:], (los[0] + d) * 64, ldims)
            if idle is None:                 # in-place (C is a merge buf)
                tmp_ap = view(tmp16[:], 0,
                              [[64, len(los)], [1, 64]])
                nc.vector.tensor_tensor(tmp_ap, lo_src, hi_src, OP.max)
                nc.vector.tensor_tensor(lo_src, lo_src, hi_src, OP.min)
                nc.vector.tensor_copy(hi_src, tmp_ap)
            else:
                N = mbufs[mcur]
                nc.vector.tensor_tensor(view(N[:], los[0] * 64, ldims),
                                        lo_src, hi_src, OP.min)
                nc.vector.tensor_tensor(view(N[:], (los[0] + d) * 64, ldims),
                                        lo_src, hi_src, OP.max)
                if idle:
                    idims = _vdims(idle)
                    nc.scalar.activation(view(N[:], idle[0] * 64, idims),
                                         view(C[:], idle[0] * 64, idims),
                                         mybir.ActivationFunctionType.Copy)
                C = N
                mcur = 1 - mcur
        # ---------------- fused final round + area_xor tail.
        # Nothing consumes the merged array except the odd-rank sum, and
        # the merge preserves the value multiset (sum(M) == r1o + r1e), so
        #   ax = odd(M) - even(M) = 2*odd(M) - (r1o + r1e)
        # and the last OEM round (d=1: min into odd ranks 1..29) collapses
        # into ONE scalar_tensor_tensor with accum_out, plus the rank-31
        # value that the round never touches.
        ar = pool.tile([128, 2], F32, tag="ar")
        nc.vector.tensor_tensor(ar[:, 0:1], ar1o[:, 0:1], ar1e[:, 0:1],
                                OP.subtract)
        nc.sync.dma_start(out=bass.AP(areas_dram.tensor, areas_dram.offset,
                                      [[2, 128], [1, 1]]),
                          in_=ar[:, 0:1])

        l31 = pool.tile([128, 1], F32, tag="l31")
        nc.vector.tensor_reduce(l31[:, 0:1], view(C[:], 31 * 64, [(1, 64)]),
                                axis=mybir.AxisListType.X, op=OP.add)
        scr2 = pool.tile([128, 1024], F16, tag="scr2")
        modd = [(128, 15), (1, 64)]
        nc.vector.scalar_tensor_tensor(
            view(scr2[:], 0, [(64, 15), (1, 64)]),
            view(C[:], 64, modd), 0.0, view(C[:], 2 * 64, modd),
            OP.add, OP.min, accum_out=ar2o[:, 0:1])
        s12 = pool.tile([128, 1], F32, tag="s12")
        nc.vector.tensor_tensor(s12[:, 0:1], ar1o[:, 0:1], ar1e[:, 0:1],
                                OP.add)
        odd = pool.tile([128, 1], F32, tag="odd")
        nc.vector.tensor_tensor(odd[:, 0:1], ar2o[:, 0:1], l31[:, 0:1],
                                OP.add)
        nc.vector.scalar_tensor_tensor(ar[:, 1:2], odd[:, 0:1], 2.0,
                                       s12[:, 0:1], OP.mult, OP.subtract)
        nc.sync.dma_start(out=bass.AP(areas_dram.tensor, areas_dram.offset + 1,
                                      [[2, 128], [1, 1]]),
                          in_=ar[:, 1:2])


_CACHED_NC = None


def _get_nc():
    global _CACHED_NC
    if _CACHED_NC is not None:
        return _CACHED_NC
    nc = bacc.Bacc("TRN2", target_bir_lowering=False, debug=False,
                   num_devices=N_CORES)
    # declaration order = input staging order: small tensors first so the
    # gather's index tile and the gt vertices land before the 2MB table
    pidx = nc.dram_tensor("pidx", [128, 1], I32, kind="ExternalInput")
    gverts = nc.dram_tensor("gverts", [128, 32], F32, kind="ExternalInput")
    table = nc.dram_tensor("table", [16384, 32], F32, kind="ExternalInput")
    areas = nc.dram_tensor("areas", [128, 2], F32, kind="ExternalOutput")
    with tile.TileContext(nc) as tc:
        _build_core_kernel(tc, areas.ap(), table.ap(), pidx.ap(), gverts.ap())
    nc.compile()
    _CACHED_NC = nc
    return nc


def kernel(output, mask, ind, target):
    global LAST_RESULTS
    output = np.asarray(output)
    mask = np.asarray(mask)
    ind = np.asarray(ind)
    target = np.asarray(target)
    B, C, H, W = output.shape

    # ---- host-side sharding (layout-only)
    in_maps = []
    for c in range(N_CORES):
        b, k0 = c // 2, 64 * (c % 2)
        table = np.ascontiguousarray(output[b].reshape(C, H * W).T).astype(F)
        idx64 = ind[b, k0:k0 + 64].astype(np.int32)
        pidx = np.tile(idx64, 2).reshape(128, 1)
        gv64 = np.ascontiguousarray(target[b, :, k0:k0 + 64].T).astype(F)
        gverts = np.tile(gv64, (2, 1))
        in_maps.append({"table": table, "pidx": pidx, "gverts": gverts})

    nc = _get_nc()
    res = run_bass_kernel_spmd(nc, in_maps, core_ids=list(range(N_CORES)))
    LAST_RESULTS = res

    # ---- host-side gather + final scalar assembly
    spg = np.zeros((B, 128), np.float32)     # area_p + area_g per poly
    ax = np.zeros((B, 128), np.float32)      # area_xor per poly
    for c in range(N_CORES):
        b, k0 = c // 2, 64 * (c % 2)
        halves = res.results[c]["areas"]     # [128, 2]
        spg[b, k0:k0 + 64] = halves[:64, 0] + halves[64:, 0]
        ax[b, k0:k0 + 64] = halves[:64, 1] + halves[64:, 1]
    inter = ((spg - ax) / 2).astype(F)
    union = ((spg + ax) / 2).astype(F)
    iou = (inter / (union + F(1e-4))).astype(F)
    mm = mask.astype(F)
    loss = F(F(1.0) - (iou * mm).sum(dtype=F) / (mm.sum(dtype=F) + F(1e-4)))
    return np.asarray(loss, dtype=np.float32)
